# revision 4
# baseline (speedup 1.0000x reference)
"""Single-head causal attention (B=4, S=4096, D=1024, H=64) on 8 trn2 cores. v3.

Sharding: core c -> batch b = c % 4, role r = c // 4; role r owns global
q-tiles / x-chunks {r, r+2, r+4, r+6} (512 rows each).

Design (cost-model driven, v3 rewrite):
- Host ships x pre-transposed and DR-folded in fp8 (chunk-major), so the
  device does no transposes and the QK/V projections run as fp8 DoubleRow
  matmuls straight off the DMA'd layout.
- K needs no bias (per-query score offsets cancel in softmax); Q's bias is
  added by a contraction-1 matmul (ones rhs) accumulated into the proj PSUM.
  V bias and the final softmax divide+transpose are applied on the host.
- Exchange: per-chunk 2-core AllGather of a 96KB packet (K fp8 DR-foldable +
  V fp8 [k,h]).
- Attention is exp-throughput-bound; the exp stream is split across BOTH
  ACT (true exp -> fp8) and DVE (Schraudolph: fp8 bits = rne(a*s + b) via
  one fp32->int8 tensor_scalar).  Masks are applied post-exp on Pool as
  int8 bitwise_and.  Slot (i, 2i+1) is diagonal-for-role1 / dead-for-role0:
  pinned to ACT with a per-core bias AP (-30000 for role0) that zeroes it,
  and computed trapezoid-shaped.
- Output: oT = [V*16; ones]^T P accumulated in PSUM per q-tile, drained as
  [65, 512] and normalized/transposed on host.
- Rows 0:127 of the sequence need bf16 precision (few softmax terms): a tiny
  bf16 special path computes them from a shared x0^T blob; host takes that
  output from role-0 cores.
"""

import math

import ml_dtypes
import numpy as np

B, S, D, H = 4, 4096, 1024, 64
NT = 4          # local chunks / q-tiles per core (512 rows each)
KC = 512
NKB = 4
SQ = 16.0       # weight prescale (Q,K,V all scaled by 16)
PSC = SQ * SQ   # score psum scale = 256
ACT_SCALE = (1.0 / 8.0) / PSC
A8 = (8.0 / math.log(2.0) / 8.0) / PSC     # 1.4427/256
B8 = 56.0 - 0.0573                          # log-centered Schraudolph bias
KILL = -30000.0
KW = 768        # packet bytes/partition per chunk: V fp8 256 | K fp8 512
VG = 80         # V group stride in Vt (64 + ones + pad to %16)

_compiled = None
TRACE = False
LAST_RESULT = None

# engine schedule: maskB slots (i, 2i+1) pinned to ACT ('A'); 16 free slots
# split 7 ACT / 9 DVE, interleaved.
_FREE = "DDADDADADDADADAD"


def _slot_engine():
    eng = {}
    k = 0
    for i in range(4):
        for j in range(2 * i + 2):
            if j == 2 * i + 1:
                eng[(i, j)] = "A"
            else:
                eng[(i, j)] = _FREE[k]
                k += 1
    return eng


SLOT_ENG = _slot_engine()


def _build():
    import concourse.bass as bass
    import concourse.mybir as mybir
    from concourse import bacc
    from concourse.tile import TileContext

    fp32 = mybir.dt.float32
    bf16 = mybir.dt.bfloat16
    fp8 = mybir.dt.float8e4
    f16 = mybir.dt.float16
    i8 = mybir.dt.int8
    i16 = mybir.dt.int16
    i32 = mybir.dt.int32
    u8 = mybir.dt.uint8
    AF = mybir.ActivationFunctionType
    DR = mybir.MatmulPerfMode.DoubleRow
    ALU = mybir.AluOpType

    nc = bacc.Bacc(None, target_bir_lowering=False)
    # inputs
    x_dr_d = nc.dram_tensor("x_dr", [128, NT * 8 * KC], fp8, kind="ExternalInput")
    x0T_d = nc.dram_tensor("x0T", [128, 8 * 128], bf16, kind="ExternalInput")
    # cst blob: wqk_dr(1024) | wv_dr(512) | bqcol+ones+bqcol0 rows... packed below
    CSTW = 1024 + 512 + 1024 + 256 + 256 + 4 + 256 + 12
    cst_d = nc.dram_tensor("cst", [128, CSTW], u8, kind="ExternalInput")
    CST2W = 2048 + 256 + 2048 + 1024
    cst2_d = nc.dram_tensor("cst2", [128, CST2W], u8, kind="ExternalInput")
    y_d = nc.dram_tensor("y", [65, NT * KC], fp32, kind="ExternalOutput")
    y0_d = nc.dram_tensor("y0", [65, 128], fp32, kind="ExternalOutput")
    q_dram = nc.dram_tensor("q_stage", [64, NT * KC], fp8)
    kv_out = [nc.dram_tensor(f"kv_out{c}", [1, 128 * KW], fp8) for c in range(NT)]
    kv_all = [nc.dram_tensor(f"kv_all{c}", [2, 128 * KW], fp8) for c in range(NT)]

    with TileContext(nc) as tc:
        with (
            tc.tile_pool(name="const", bufs=1) as cpool,
            tc.tile_pool(name="pX", bufs=4) as ppool,
            tc.tile_pool(name="fin", bufs=2) as fpool,
            tc.tile_pool(name="psS", bufs=3, space="PSUM") as psS,    # 2 banks x3
            tc.tile_pool(name="psO", bufs=2, space="PSUM") as psO,    # 1 bank x2
        ):
            # ---- persistent SBUF ----
            x_dr = cpool.tile([128, NT * 8 * KC], fp8, tag="x_dr")
            xdr5 = x_dr.rearrange("p (c g s q) -> p c g s q", c=NT, g=4, s=2)
            x0T = cpool.tile([128, 8 * 128], bf16, tag="x0T")
            cst = cpool.tile([128, CSTW], u8, tag="cst")
            off = 0
            wqk_dr = cst[:, off:off + 1024].bitcast(fp8).rearrange(
                "p (g s m) -> p g s m", g=4, s=2); off += 1024
            wv_dr = cst[:, off:off + 512].bitcast(fp8).rearrange(
                "p (g s h) -> p g s h", g=4, s=2); off += 512
            onesr = cst[0:1, off:off + 1024].bitcast(bf16); off += 1024   # [1,512]
            brow = cst[0:1, off:off + 256].bitcast(bf16); off += 256      # [1,128]: 16bq|0
            brow0 = cst[0:1, off:off + 256].bitcast(bf16); off += 256     # [1,128]: bq|0
            killAP = cst[:, off:off + 4].bitcast(fp32); off += 4          # [128,1]
            maskB = cst[:, off:off + 256].bitcast(i8); off += 256         # [128,2,128]
            cst2 = cpool.tile([128, CST2W], u8, tag="cst2")
            off = 0
            maskA = cst2[:, off:off + 2048].bitcast(i8); off += 2048      # [128,4,512]
            tri16 = cst2[:, off:off + 256].bitcast(i16); off += 256       # [128,128]
            wqk0 = cst2[:, off:off + 2048].bitcast(bf16); off += 2048     # [128,8*128]
            wv0 = cst2[:, off:off + 1024].bitcast(bf16); off += 1024      # [128,8*64]

            KTfc = [cpool.tile([32, 2 * 2 * KC], fp8, name=f"KTf{c}", tag=f"KTf{c}") for c in range(NT)]
            KTf3c = [t.rearrange("p (g k) -> p g k", g=2) for t in KTfc]
            QTf = cpool.tile([32, 2 * NT * KC], fp8, tag="QTf")
            QTf3 = QTf.rearrange("p (g q) -> p g q", g=2)
            Vtc = [cpool.tile([128, 2 * NKB * VG], fp8, name=f"Vt{c}", tag=f"Vt{c}") for c in range(NT)]
            Vt3c = [t.rearrange("p (n s) -> p n s", s=VG) for t in Vtc]
            qtmp = cpool.tile([64, NT * KC], fp8, tag="qtmp")
            kvst = cpool.tile([128, NT * KW], fp8, tag="kvst")
            ysb = cpool.tile([65, NT * KC], fp32, tag="ysb")
            qk0 = cpool.tile([128, 128], bf16, tag="qk0")
            probs0 = cpool.tile([128, 128], f16, tag="probs0")
            v0a = cpool.tile([128, 65], f16, tag="v0a")
            y0sb = cpool.tile([65, 128], fp32, tag="y0sb")

            # ---- loads (x0+cst first: they gate proj0) ----
            nc.sync.dma_start(out=cst[:], in_=cst_d[:])
            nc.sync.dma_start(
                out=x_dr[:, 0:8 * KC], in_=x_dr_d[:, 0:8 * KC])
            for c in range(1, NT):
                nc.sync.dma_start(
                    out=x_dr[:, c * 8 * KC:(c + 1) * 8 * KC],
                    in_=x_dr_d[:, c * 8 * KC:(c + 1) * 8 * KC])

            # ones column of V groups (fp8 1.0 = 0x38 = 56)
            for c in range(NT):
                nc.vector.memset(Vt3c[c][:, :, H:H + 1].bitcast(u8), 56)
            nc.gpsimd.memset(v0a[:, H:H + 1], 1.0)
            warm = cpool.tile([128, 1], fp32, tag="warm")
            nc.vector.memset(warm[:], 0.0)
            nc.scalar.activation(warm[:], warm[:], AF.Exp, scale=1.0)

            # ---- projections + exchange ----
            def project(c):
                psP = psS.tile([128, 2 * KC], fp32, tag="psH")
                psQK = psP[:, 0:KC]
                psVt = psP[:, KC:KC + NKB * H]
                for g in range(4):
                    nc.tensor.matmul(
                        psQK, wqk_dr[:, g, :, :], xdr5[:, c, g, :, :],
                        start=(g == 0), stop=(g == 3), perf_mode=DR,
                        skip_group_check=True)
                nc.tensor.matmul(psQK[0:64, :], brow[0:1, 0:64], onesr[:],
                                 start=False, stop=True, skip_group_check=True)
                # K -> packet (ACT) as soon as QK group lands
                nc.scalar.activation(kvst[64:128, c * KW + 256:c * KW + KW],
                                     psQK[64:128, :], AF.Copy)
                for kb in range(NKB):
                    for g in range(4):
                        nc.tensor.matmul(
                            psVt[:, kb * H:(kb + 1) * H],
                            xdr5[:, c, g, :, kb * 128:(kb + 1) * 128],
                            wv_dr[:, g, :, :],
                            start=(g == 0), stop=(g == 3), perf_mode=DR,
                            skip_group_check=True)
                nc.vector.tensor_copy(kvst[:, c * KW:c * KW + 256], psVt)
                # Q staging (only gates local attention via qfold)
                nc.vector.tensor_copy(qtmp[:, c * KC:(c + 1) * KC], psQK[0:64, :])

            def send(c):
                nc.sync.dma_start(
                    out=kv_out[c][:].rearrange("o (p w) -> (o p) w", w=KW),
                    in_=kvst[:, c * KW:(c + 1) * KW])
                nc.gpsimd.collective_compute(
                    "AllGather", mybir.AluOpType.bypass,
                    replica_groups=[[0, 4], [1, 5], [2, 6], [3, 7]],
                    ins=[kv_out[c][:]], outs=[kv_all[c][:]])

            def unpack(c):
                kvv = kv_all[c][:].rearrange("r (p w) -> r p w", w=KW)
                for r in range(2):
                    nc.sync.dma_start(
                        out=KTf3c[c][:, :, r * KC:(r + 1) * KC],
                        in_=kvv[r, 64:128, 256:KW]
                            .rearrange("(g p) s -> p g s", g=2))
                    nc.sync.dma_start(
                        out=Vt3c[c][:, r * NKB:(r + 1) * NKB, 0:H],
                        in_=kvv[r, :, 0:256].rearrange("k (n h) -> k n h", h=H))

            def qfold(lo, hi):
                nc.sync.dma_start(out=q_dram[:, lo * KC:hi * KC],
                                  in_=qtmp[:, lo * KC:hi * KC])
                nc.sync.dma_start(
                    out=QTf3[:, :, lo * KC:hi * KC],
                    in_=q_dram[:, lo * KC:hi * KC]
                        .rearrange("(g p) q -> p g q", g=2))

            nc.sync.dma_start(out=x0T[:], in_=x0T_d[:])
            project(0)
            send(0)
            nc.sync.dma_start(out=cst2[:], in_=cst2_d[:])
            project(1)
            send(1)
            qfold(0, 2)
            unpack(0)
            project(2)
            send(2)
            unpack(1)
            project(3)
            send(3)
            qfold(2, 4)
            unpack(2)
            unpack(3)

            # ---- special path: global rows 0:127 in bf16 ----
            def special():
                wqk03 = wqk0.rearrange("p (d m) -> p d m", d=8)
                x0T3 = x0T.rearrange("p (d q) -> p d q", d=8)
                psQ0f = psS.tile([128, 2 * KC], fp32, name="psQ0f", tag="psH")
                psQ0 = psQ0f[0:64, 0:128]
                for db in range(8):
                    nc.tensor.matmul(
                        psQ0, wqk03[:, db, 0:64], x0T3[:, db, :],
                        start=(db == 0), stop=False, skip_group_check=True)
                nc.tensor.matmul(psQ0, brow0[0:1, 0:64], onesr[:, 0:128],
                                 start=False, stop=True, skip_group_check=True)
                nc.vector.tensor_copy(qk0[0:64, :], psQ0)
                psK0f = psS.tile([128, 2 * KC], fp32, name="psK0f", tag="psH")
                psK0 = psK0f[0:64, 0:128]
                for db in range(8):
                    nc.tensor.matmul(
                        psK0, wqk03[:, db, 64:128], x0T3[:, db, :],
                        start=(db == 0), stop=(db == 7), skip_group_check=True)
                nc.vector.tensor_copy(qk0[0:64, :].bitcast(bf16), psK0[:]) if False else None
                k0sb = fpool.tile([64, 128], bf16, tag="k0sb")
                nc.vector.tensor_copy(k0sb[:], psK0)
                psS0f = psS.tile([128, 2 * KC], fp32, name="psS0f", tag="psH")
                psS0 = psS0f[:, 0:128]
                nc.tensor.matmul(psS0, k0sb[:], qk0[0:64, :],
                                 start=True, stop=True, skip_group_check=True)
                nc.scalar.activation(probs0[:], psS0, AF.Exp, scale=0.125)
                nc.vector.tensor_tensor(probs0.bitcast(i32)[:], probs0.bitcast(i32)[:],
                                        tri16.bitcast(i32)[:], op=ALU.bitwise_and)
                psV0f = psS.tile([128, 2 * KC], fp32, name="psV0f", tag="psH")
                psV0 = psV0f[:, 0:H]
                for db in range(8):
                    nc.tensor.matmul(
                        psV0, x0T.rearrange("p (d q) -> p d q", d=8)[:, db, :],
                        wv0.rearrange("p (d h) -> p d h", d=8)[:, db, :],
                        start=(db == 0), stop=(db == 7), skip_group_check=True)
                nc.vector.tensor_copy(v0a[:, 0:H], psV0)
                psO0f = psS.tile([128, 2 * KC], fp32, name="psO0f", tag="psH")
                psO0 = psO0f[0:65, 0:128]
                nc.tensor.matmul(psO0, v0a[:], probs0[:], start=True, stop=True,
                                 skip_group_check=True)
                nc.vector.tensor_copy(y0sb[:], psO0)
                nc.gpsimd.dma_start(out=y0_d[:], in_=y0sb[:])

            # ---- attention ----
            # ---- attention: flat half-slot pipeline, PE stream skewed so
            # scores(k+1) are emitted before PV(k) (avoids PE.SEQ head-block
            # behind the exp dependency) ----
            halves = []
            for i in range(4):
                for j in range(2 * i + 2):
                    for h in range(2):
                        halves.append((i, j, h))
            oTs = {}
            firsts = {}
            state = {}

            def emit_scores(k):
                i, j, h = halves[k]
                maskb = (j == 2 * i + 1)
                if (i, j, h) == (i, 0, 0) and h == 0 and j == 0:
                    oTs[i] = psO.tile([65, KC], fp32, name=f"oT{i}", tag="oT")
                    firsts[i] = True
                psH = psS.tile([128, 2 * KC], fp32, tag="psH")
                psH3 = psH.rearrange("p (n q) -> p n q", q=KC)
                pXh = ppool.tile([128, 2 * KC], fp8, tag="pXh")
                for kbl in range(2):
                    kb = 2 * h + kbl
                    qoff = kb * 128 if maskb else 0
                    n = KC - qoff
                    nc.tensor.matmul(
                        psH3[:, kbl, 0:n],
                        KTf3c[j // 2][:, :, (j % 2) * KC + kb * 128:
                                      (j % 2) * KC + (kb + 1) * 128],
                        QTf3[:, :, i * KC + qoff:(i + 1) * KC],
                        start=True, stop=True, perf_mode=DR,
                        skip_group_check=True)
                state[k] = (psH3, pXh)

            def emit_rest(k):
                i, j, h = halves[k]
                eng = SLOT_ENG[(i, j)]
                maskb = (j == 2 * i + 1)
                maska = (j == 2 * i)
                psH3, pXh = state.pop(k)
                oT = oTs[i]
                ncols = KC if not maskb else (KC if h == 0 else 256)
                if eng == "A":
                    bias = killAP[:, 0:1] if maskb else 0.0
                    nc.scalar.activation(
                        pXh.rearrange("p (n q) -> p n q", q=KC)[:, :, 0:ncols],
                        psH3[:, :, 0:ncols], AF.Exp, scale=ACT_SCALE,
                        bias=bias)
                else:
                    nc.vector.tensor_scalar(
                        pXh.bitcast(i8).rearrange("p (n q) -> p n q", q=KC)
                           [:, :, 0:ncols],
                        psH3[:, :, 0:ncols], A8, B8,
                        op0=ALU.mult, op1=ALU.add)
                pXf = pXh.rearrange("p (n q) -> p n q", q=KC)
                pX32 = pXh.bitcast(i32)
                if maska:
                    nc.vector.tensor_tensor(
                        pX32[:], pX32[:],
                        maskA.bitcast(i32)[:, h * 256:(h + 1) * 256],
                        op=ALU.bitwise_and)
                if maskb:
                    pX32t = pX32.rearrange("p (n q) -> p n q", q=128)
                    nc.vector.tensor_tensor(
                        pX32t[:, :, 0:32], pX32t[:, :, 0:32],
                        maskB.bitcast(i32).rearrange("p (n q) -> p n q", q=32)[:],
                        op=ALU.bitwise_and)
                if maskb:
                    for kbl in range(2):
                        kb = 2 * h + kbl
                        n = KC - kb * 128
                        nc.tensor.matmul(
                            oT[:, kb * 128:KC],
                            Vt3c[j // 2][:, (j % 2) * NKB + kb, 0:H + 1],
                            pXf[:, kbl, 0:n],
                            start=False, stop=(h == 1 and kbl == 1),
                            skip_group_check=True)
                else:
                    nc.tensor.matmul(
                        oT[:],
                        Vt3c[j // 2][:, (j % 2) * NKB + 2 * h:
                                     (j % 2) * NKB + 2 * h + 2, 0:H + 1],
                        pXf[:],
                        start=firsts[i], stop=False, perf_mode=DR,
                        skip_group_check=True)
                    firsts[i] = False
                if maskb and h == 1:
                    nc.scalar.activation(ysb[:, i * KC:(i + 1) * KC], oT[:],
                                         AF.Copy)

            special()
            emit_scores(0)
            emit_scores(1)
            for k in range(2, len(halves)):
                emit_scores(k)
                emit_rest(k - 2)
            emit_rest(len(halves) - 2)
            emit_rest(len(halves) - 1)
            nc.sync.dma_start(out=y_d[:], in_=ysb[:])

    nc.compile()
    return nc


def _tri_block():
    p = np.arange(128)[:, None]
    q = np.arange(128)[None, :]
    return (q >= p)


def _host_consts(role):
    # maskA [128, 4, 512] int8: role0 = causal blocks, role1 = keep-all
    keep = np.zeros((128, 4, 512), dtype=np.uint8)
    if role == 1:
        keep[:] = 0xFF
    else:
        tri = _tri_block()
        for kb in range(4):
            qb = np.arange(512)[None, :] // 128
            k = (qb > kb).astype(np.uint8) * 0xFF
            blk = k.repeat(128, axis=0)
            blk[:, kb * 128:(kb + 1) * 128] = tri.astype(np.uint8) * 0xFF
            keep[:, kb, :] = blk
    maskA = keep.reshape(128, 2048)
    # maskB [128, 2, 128]: role1 = tri, role0 = zeros
    if role == 1:
        mb = (_tri_block().astype(np.uint8) * 0xFF)
    else:
        mb = np.zeros((128, 128), dtype=np.uint8)
    maskB = np.concatenate([mb, mb], axis=1)
    tri16 = np.where(_tri_block(), np.uint16(0xFFFF), np.uint16(0)).astype(np.uint16)
    kill = np.full((128, 1), KILL if role == 0 else 0.0, dtype=np.float32)
    return maskA, maskB, tri16, kill


def kernel(x, Wq_w, Wq_b, Wk_w, Wk_b, Wv_w, Wv_b):
    global _compiled, LAST_RESULT
    from concourse.bass_utils import run_bass_kernel_spmd

    x = np.asarray(x, dtype=np.float32)
    Wq_w = np.asarray(Wq_w, dtype=np.float32)
    Wq_b = np.asarray(Wq_b, dtype=np.float32)
    Wk_w = np.asarray(Wk_w, dtype=np.float32)
    Wv_w = np.asarray(Wv_w, dtype=np.float32)
    Wv_b = np.asarray(Wv_b, dtype=np.float32)

    fp8 = ml_dtypes.float8_e4m3
    bf = ml_dtypes.bfloat16

    wcat = np.concatenate([Wq_w, Wk_w], axis=1)           # [1024, 128]
    wqk_dr = (wcat * SQ).reshape(4, 2, 128, 128).transpose(2, 0, 1, 3) \
        .reshape(128, 1024).astype(fp8)
    wv_dr = (Wv_w * SQ).reshape(4, 2, 128, H).transpose(2, 0, 1, 3) \
        .reshape(128, 512).astype(fp8)
    # cst filled per-core below (killAP/maskB are role-dependent)

    wqk0 = np.ascontiguousarray(
        wcat.reshape(8, 128, 128).transpose(1, 0, 2).reshape(128, 1024)).astype(bf)
    wv0 = np.ascontiguousarray(
        Wv_w.reshape(8, 128, H).transpose(1, 0, 2).reshape(128, 512)).astype(bf)
    brow = np.zeros((1, 128), dtype=bf)
    brow[0, 0:64] = (Wq_b * SQ).astype(bf)
    brow0 = np.zeros((1, 128), dtype=bf)
    brow0[0, 0:64] = Wq_b.astype(bf)
    onesr = np.ones((1, 512), dtype=bf)

    if _compiled is None:
        _compiled = _build()
    nc = _compiled

    in_maps = []
    for c in range(8):
        b, role = c % 4, c // 4
        maskA, maskB, tri16, kill = _host_consts(role)
        cst = np.concatenate([
            wqk_dr.view(np.uint8), wv_dr.view(np.uint8),
            np.broadcast_to(onesr.view(np.uint8), (128, 1024)),
            np.broadcast_to(brow.view(np.uint8), (128, 256)),
            np.broadcast_to(brow0.view(np.uint8), (128, 256)),
            kill.view(np.uint8),
            maskB.view(np.uint8),
            np.zeros((128, 12), dtype=np.uint8),
        ], axis=1)
        xl = x[b].reshape(NT * 2, KC, D)[role::2].reshape(NT, KC, D)
        # x_dr[p, c, g, s, q] = xl[c, q, g*256 + s*128 + p]
        xf = xl.reshape(NT, KC, 4, 2, 128).transpose(4, 0, 2, 3, 1)
        x_dr = np.ascontiguousarray(xf).reshape(128, NT * 8 * KC).astype(fp8)
        x0T = np.ascontiguousarray(
            x[b][0:128, :].T.reshape(8, 128, 128).transpose(1, 0, 2)
            .reshape(128, 1024)).astype(bf)
        cst2 = np.concatenate([
            maskA.view(np.uint8),
            tri16.view(np.uint8).reshape(128, 256),
            wqk0.view(np.uint8).reshape(128, 2048),
            wv0.view(np.uint8).reshape(128, 1024),
        ], axis=1)
        in_maps.append({"x_dr": x_dr, "x0T": x0T,
                        "cst": np.ascontiguousarray(cst),
                        "cst2": np.ascontiguousarray(cst2)})

    kw = {}
    if TRACE:
        kw = dict(trace=True, trace_cores=list(range(8)))
    res = run_bass_kernel_spmd(nc, in_maps, core_ids=list(range(8)), **kw)
    LAST_RESULT = res

    out = np.empty((B, S, H), dtype=np.float32)
    for c in range(8):
        b, role = c % 4, c // 4
        y = res.results[c]["y"]            # [65, NT*KC]
        num = y[0:64, :].T / SQ            # [2048, 64]
        den = y[64, :][:, None]
        yt = num / den + Wv_b[None, :]
        for i in range(NT):
            g = 2 * i + role
            out[b, g * KC:(g + 1) * KC, :] = yt[i * KC:(i + 1) * KC, :]
        if role == 0:
            y0 = res.results[c]["y0"]      # [65, 128]
            out[b, 0:128, :] = y0[0:64, :].T / y0[64, :][:, None] + Wv_b[None, :]
    return out
